# revision 18
# baseline (speedup 1.0000x reference)
"""Trainium2 Bass kernel: batched time-domain cross-correlation.

Computes, for each of 2048 (=64x32) independent pairs (fp32):
    out[g, l] = sum_k d1[g, k + l - 301] * d2[g, k],   l in [0, 603)

Algorithm: overlap-save block correlation in a half-shift (negacyclic)
real-DFT basis, so every matmul has a *shared* stationary operand (the
transform matrices) and batches all pairs in the moving operand:

  xp = d1 zero-padded/shifted; y = d2 zero-padded.
  out[B*c + j] = sum_v corr(w_{v+c}, y_v)[j]     (j in [0, B))
    w_s = xp[B*s : B*s + 2B]  (windows, stride B, length N=2B)
    y_v = y[B*v : B*v + B]    (blocks, zero-padded to N)
  Per-block circular corr via length-N negacyclic real DFT:
    bins k: Ur[k] = sum_n u[n] cos(pi n (2k+1)/N)
            Ui[k] = -sum_n u[n] sin(pi n (2k+1)/N),  k in [0, B)
    Z = X * conj(Y):  Zr = XrYr + XiYi ; Zi = XiYr - XrYi
    z[0:B] = Minv @ [Zr; Zi]  (exact: aliasing only corrupts j > B)

Mapping: forward transforms + inverse are PE matmuls with shared
stationaries; the pointwise spectral products run on the Vector engine
with the v-sum done by segmented tensor_reduce.

Sharding: data-parallel over the 2048 pairs, 256 pairs per core, 8 cores.
"""

import math
import os
import sys

import ml_dtypes
import numpy as np

if "/opt/trn_rl_repo" not in sys.path:  # harness safety; axon site usually set
    sys.path.insert(0, "/opt/trn_rl_repo")

import concourse.bacc as bacc
import concourse.bass as bass
import concourse.mybir as mybir
import concourse.tile as tile
from concourse.bass_utils import run_bass_kernel_spmd

# ---- problem constants (hardcoded per contest contract) ----
NB_PAIRS, NCH, NT = 64, 32, 3000
LAGS = 603
SHIFT = 301  # NLAG + 1
NCORES = 8
G = (NB_PAIRS * NCH) // NCORES  # 256 pairs per core

# ---- tunables ----
B = int(os.environ.get("KB", "384"))  # lag/block granularity; N = 2B
GH = int(os.environ.get("KGH", "64"))  # pairs per g-chunk (SBUF working set)
# of every RED_FRAC product/tree ops, GP_FRAC go to GpSimd and the rest to DVE
GP_FRAC = int(os.environ.get("KGP", "1"))
RED_FRAC = int(os.environ.get("KRF", "4"))
DT_MM = mybir.dt.bfloat16  # matmul moving/stationary dtype
DT_Z = mybir.dt.bfloat16  # spectra / reduced-product dtype
DT_VE = mybir.dt.bfloat16  # elementwise product dtype (2x DVE rate)
NP_MM = ml_dtypes.bfloat16

# derived
N = 2 * B
V = math.ceil(NT / B)  # y blocks
C = math.ceil(LAGS / B)  # output lag blocks
S = V + C - 1  # x windows
SP = S  # no padding needed for bf16 matmuls
CP = C
BS = B // 128  # 128-chunks per B
NQ = N // 128  # contraction chunks of a full window
NJ = B // 128  # 128-chunks of B (bins halves / out j groups)
NR = 2 * NJ  # psum bin groups of the spectrum
U = (SP - 1) * BS + NQ  # 128-chunks in xp (covers padded windows)
NBB = U * 128
W = (V * B) // 128  # 128-chunks in y
# uneven pair-chunks sized so each chunk's x-fwd psum group fits one bank
# (gh*SP <= 512 free fp32) -> one matmul group per (r, chunk): fewest PE instrs
GHX = 512 // SP  # 56 for SP=9
_chunks = []
_g = 0
while _g < G:
    _chunks.append((_g, min(GHX, G - _g)))
    _g += min(GHX, G - _g)
GHMAX = max(gh for _, gh in _chunks)
FG = 128  # pairs per inverse group
assert G % FG == 0

_PE_CACHE = {}
LAST_EXEC_NS = None
LAST_TRACE = None


def _matrices():
    n = np.arange(N, dtype=np.float64)[:, None]
    k = np.arange(B, dtype=np.float64)[None, :]
    theta = np.pi * n * (2 * k + 1) / N
    ffull = np.concatenate([np.cos(theta), -np.sin(theta)], axis=1)  # [N, 2B]
    minv = np.linalg.inv(ffull.T)[:B, :]  # [B, 2B]
    return ffull.astype(np.float32), minv.astype(np.float32)


def _const_tiles():
    """FW [128, NR*NQ*128]: FW[i, ((r*NQ)+q)*128 + col] = Ffull[128q+i, 128r+col]
    (r-major so each r's blocks are one contiguous DMA piece)
    MT [128, 3*NJ*NJ*128]: for zg in {Mr, Mi, -Mi}:
        MT[i, ((zg*NJ + rh)*NJ + jg)*128 + col] = M[128jg + col, 128rh + i]
    """
    ffull, minv = _matrices()
    fw = np.zeros((128, NR * NQ * 128), dtype=np.float32)
    for q in range(NQ):
        for r in range(NR):
            fw[:, (r * NQ + q) * 128 : (r * NQ + q + 1) * 128] = ffull[
                128 * q : 128 * (q + 1), 128 * r : 128 * (r + 1)
            ]
    mr = minv[:, :B]
    mi = minv[:, B:]
    mats = [mr, mi, -mi]
    mt = np.zeros((128, 3 * NJ * NJ * 128), dtype=np.float32)
    for zg in range(3):
        for rh in range(NJ):
            for jg in range(NJ):
                blk = mats[zg][128 * jg : 128 * (jg + 1), 128 * rh : 128 * (rh + 1)]
                base = ((zg * NJ + rh) * NJ + jg) * 128
                mt[:, base : base + 128] = blk.T
    return fw, mt


def build_kernel():
    nc = bacc.Bacc(
        "TRN2",
        target_bir_lowering=False,
        debug=False,
        num_devices=NCORES,
    )

    xp_d = nc.dram_tensor("xp", [128, G, U], DT_MM, kind="ExternalInput")
    yp_d = nc.dram_tensor("yp", [128, G, W], DT_MM, kind="ExternalInput")
    fw_d = nc.dram_tensor("fw", [128, NR * NQ * 128], DT_MM, kind="ExternalInput")
    mt_d = nc.dram_tensor("mt", [128, 3 * NJ * NJ * 128], DT_Z, kind="ExternalInput")
    out_d = nc.dram_tensor("out", [128, G, NJ, C], mybir.dt.float32,
                           kind="ExternalOutput")

    with tile.TileContext(nc, trace_sim=False) as tc:
        with (
            tc.tile_pool(name="const", bufs=1) as cpool,
            tc.tile_pool(name="io", bufs=2) as iopool,
            tc.tile_pool(name="spec", bufs=2) as spool,
            tc.tile_pool(name="work", bufs=3) as wpool,
            tc.tile_pool(name="zpool", bufs=1) as zpool,
            tc.tile_pool(name="psum", bufs=1, space=bass.MemorySpace.PSUM) as ppool,
        ):
            fw_t = cpool.tile([128, NR * NQ * 128], DT_MM, tag="fw")
            mt_t = cpool.tile([128, 3 * NJ * NJ * 128], DT_Z, tag="mt")
            zr = zpool.tile([128, NJ, G, CP], DT_Z, tag="zr")
            zi = zpool.tile([128, NJ, G, 2, CP], DT_Z, tag="zi")
            if CP > C:
                nc.gpsimd.memset(zr[:, :, :, C:], 0.0)
                nc.gpsimd.memset(zi[:, :, :, :, C:], 0.0)

            tt_i = 0

            def tt_eng(gpf, rdf):
                # weighted DVE/GpSimd split over all product/add ops:
                # gpf of every rdf ops go to GpSimd, rest to DVE
                nonlocal tt_i
                tt_i += 1
                if rdf and (tt_i - 1) % rdf < gpf:
                    return nc.gpsimd
                return nc.vector

            outt = iopool.tile([128, G, NJ, C], mybir.dt.float32, tag="outt", bufs=1)

            def emit_inverse(fgi):
                gsl = slice(fgi * FG, (fgi + 1) * FG)
                for jg in range(NJ):
                    ps = ppool.tile([128, FG, CP], mybir.dt.float32, tag="psC",
                                    bufs=2)
                    nmm = 3 * NJ
                    i = 0
                    for rh in range(NJ):
                        srcs = (
                            (0, zr[:, rh, gsl, :]),
                            (1, zi[:, rh, gsl, 0, :]),
                            (2, zi[:, rh, gsl, 1, :]),
                        )
                        for zg, rhs in srcs:
                            lhsT = mt_t[
                                :,
                                ((zg * NJ + rh) * NJ + jg) * 128 :
                                ((zg * NJ + rh) * NJ + jg + 1) * 128,
                            ]
                            nc.tensor.matmul(
                                ps[:], lhsT, rhs,
                                start=(i == 0), stop=(i == nmm - 1),
                            )
                            i += 1
                    nc.scalar.copy(out=outt[:, gsl, jg, :], in_=ps[:, :, :C])

            inv_emitted = 0
            for ci, (g0, gh) in enumerate(_chunks):
                last = ci == len(_chunks) - 1
                xin = iopool.tile([128, GHMAX, U], DT_MM, tag="xin", bufs=3)
                yin = iopool.tile([128, GHMAX, W], DT_MM, tag="yin", bufs=3)
                nc.sync.dma_start(xin[:, :gh, :], xp_d.ap()[:, g0 : g0 + gh, :])
                nc.sync.dma_start(yin[:, :gh, :], yp_d.ap()[:, g0 : g0 + gh, :])
                if ci == 1:
                    # mt is first needed by the first deferred inverse
                    nc.sync.dma_start(mt_t[:], mt_d.ap())
                if ci == 0:
                    # consts after the first input tiles: r-pieces in use order
                    r_order0 = [x for rh in range(NJ) for x in (rh, NJ + rh)]
                    for r in r_order0:
                        nc.sync.dma_start(
                            fw_t[:, r * NQ * 128 : (r + 1) * NQ * 128],
                            fw_d.ap()[:, r * NQ * 128 : (r + 1) * NQ * 128],
                        )

                xs = spool.tile([128, NR, GHMAX, SP], DT_VE, tag="xs")
                ys = spool.tile([128, NR, GHMAX, V], DT_VE, tag="ys")

                # ---- forward transforms, x and y interleaved per bin
                # group; r-order pairs (rh, NJ+rh) so PW group rh unblocks
                # after two r-iterations
                r_order = [x for rh in range(NJ) for x in (rh, NJ + rh)]
                for r in r_order:
                    ps = ppool.tile([128, GHMAX, SP], mybir.dt.float32, tag="psA",
                                    bufs=4)
                    for q in range(NQ):
                        lhsT = fw_t[:, (r * NQ + q) * 128 : (r * NQ + q + 1) * 128]
                        rhs = xin[:, 0:gh, q : q + BS * (SP - 1) + 1 : BS]
                        nc.tensor.matmul(
                            ps[:, :gh, :], lhsT, rhs,
                            start=(q == 0), stop=(q == NQ - 1),
                        )
                    nc.scalar.copy(out=xs[:, r, 0:gh, :], in_=ps[:, :gh, :])
                    ps = ppool.tile([128, GHMAX, V], mybir.dt.float32, tag="psB",
                                    bufs=2)
                    for q in range(NJ):
                        lhsT = fw_t[:, (r * NQ + q) * 128 : (r * NQ + q + 1) * 128]
                        rhs = yin[:, 0:gh, q : q + BS * (V - 1) + 1 : BS]
                        nc.tensor.matmul(
                            ps[:, :gh, :], lhsT, rhs,
                            start=(q == 0), stop=(q == NJ - 1),
                        )
                    nc.scalar.copy(out=ys[:, r, 0:gh, :], in_=ps[:, :gh, :])

                # deferred inverse: emit groups whose products finished in
                # prior chunks AFTER this chunk's forward matmuls, so the PE
                # queue never stalls waiting on the product engines
                while (inv_emitted + 1) * FG <= g0:
                    emit_inverse(inv_emitted)
                    inv_emitted += 1

                # ---- pointwise products + v-sum tree (DVE + GpSimd) ----
                # bias the last chunk toward DVE (faster) to shrink the tail
                gpf, rdf = (1, 8) if last else (GP_FRAC, RED_FRAC)
                HV = V // 2
                for c in range(C):
                    for rh in range(NJ):
                        with nc.allow_low_precision("bf16 spectra products"):
                            pr = wpool.tile([128, GHMAX, 2, V], DT_VE, tag="pr",
                                            bufs=4)
                            tt_eng(gpf, rdf).tensor_mul(
                                pr[:, :gh, 0, :],
                                xs[:, rh, :gh, c : c + V],
                                ys[:, rh, :gh, :],
                            )
                            tt_eng(gpf, rdf).tensor_mul(
                                pr[:, :gh, 1, :],
                                xs[:, NJ + rh, :gh, c : c + V],
                                ys[:, NJ + rh, :gh, :],
                            )
                            # tree-sum over (2, V): stride-1 halves each pass
                            w4 = wpool.tile([128, GHMAX, 2, HV], DT_VE, tag="w4",
                                            bufs=4)
                            tt_eng(gpf, rdf).tensor_add(
                                w4[:, :gh], pr[:, :gh, :, 0:HV],
                                pr[:, :gh, :, HV:V],
                            )
                            w2 = wpool.tile([128, GHMAX, 2, HV // 2], DT_VE,
                                            tag="w2", bufs=4)
                            tt_eng(gpf, rdf).tensor_add(
                                w2[:, :gh], w4[:, :gh, :, 0 : HV // 2],
                                w4[:, :gh, :, HV // 2 : HV],
                            )
                            w1 = wpool.tile([128, GHMAX, 2], DT_VE, tag="w1",
                                            bufs=4)
                            tt_eng(gpf, rdf).tensor_add(
                                w1[:, :gh], w2[:, :gh, :, 0], w2[:, :gh, :, 1]
                            )
                            tt_eng(gpf, rdf).tensor_add(
                                zr[:, rh, g0 : g0 + gh, c], w1[:, :gh, 0],
                                w1[:, :gh, 1],
                            )
                            pr2 = wpool.tile([128, GHMAX, 2, V], DT_VE, tag="pr",
                                             bufs=4)
                            tt_eng(gpf, rdf).tensor_mul(
                                pr2[:, :gh, 0, :],
                                xs[:, NJ + rh, :gh, c : c + V],
                                ys[:, rh, :gh, :],
                            )
                            tt_eng(gpf, rdf).tensor_mul(
                                pr2[:, :gh, 1, :],
                                xs[:, rh, :gh, c : c + V],
                                ys[:, NJ + rh, :gh, :],
                            )
                            m4 = wpool.tile([128, GHMAX, 2, HV], DT_VE, tag="w4",
                                            bufs=4)
                            tt_eng(gpf, rdf).tensor_add(
                                m4[:, :gh], pr2[:, :gh, :, 0:HV],
                                pr2[:, :gh, :, HV:V],
                            )
                            m2 = wpool.tile([128, GHMAX, 2, HV // 2], DT_VE,
                                            tag="w2", bufs=4)
                            tt_eng(gpf, rdf).tensor_add(
                                m2[:, :gh], m4[:, :gh, :, 0 : HV // 2],
                                m4[:, :gh, :, HV // 2 : HV],
                            )
                            tt_eng(gpf, rdf).tensor_add(
                                zi[:, rh, g0 : g0 + gh, :, c], m2[:, :gh, :, 0],
                                m2[:, :gh, :, 1],
                            )

            while inv_emitted < G // FG:
                emit_inverse(inv_emitted)
                inv_emitted += 1
            nc.sync.dma_start(out_d.ap()[:], outt[:])

    nc.compile()
    return nc


def _prep_core_inputs(d1f, d2f, fw, mt, core):
    """d1f/d2f: [2048, 3000] fp32. Returns the in_map for `core`."""
    sl = slice(core * G, (core + 1) * G)
    x = d1f[sl]
    y = d2f[sl]
    xp = np.zeros((G, NBB), dtype=np.float32)
    xp[:, SHIFT : SHIFT + NT] = x
    yp = np.zeros((G, V * B), dtype=np.float32)
    yp[:, :NT] = y
    # device layouts: xpT[p, g, u] = xp[g, 128u + p]
    xpT = np.ascontiguousarray(xp.reshape(G, U, 128).transpose(2, 0, 1)).astype(NP_MM)
    ypT = np.ascontiguousarray(yp.reshape(G, W, 128).transpose(2, 0, 1)).astype(NP_MM)
    return {"xp": xpT, "yp": ypT, "fw": fw.astype(NP_MM), "mt": mt.astype(NP_MM)}


def kernel(data1: np.ndarray, data2: np.ndarray) -> np.ndarray:
    import time

    d1f = np.ascontiguousarray(data1, dtype=np.float32).reshape(-1, NT)
    d2f = np.ascontiguousarray(data2, dtype=np.float32).reshape(-1, NT)
    fw, mt = _const_tiles()

    t0 = time.time()
    if "nc" not in _PE_CACHE:
        _PE_CACHE["nc"] = build_kernel()
    nc = _PE_CACHE["nc"]
    print(f"[kernel] build+compile {time.time() - t0:.1f}s", file=sys.stderr,
          flush=True)

    in_maps = [_prep_core_inputs(d1f, d2f, fw, mt, i) for i in range(NCORES)]
    t0 = time.time()
    res = run_bass_kernel_spmd(nc, in_maps, core_ids=list(range(NCORES)))
    print(f"[kernel] spmd run {time.time() - t0:.1f}s", file=sys.stderr, flush=True)
    global LAST_EXEC_NS, LAST_TRACE
    LAST_EXEC_NS = res.exec_time_ns
    LAST_TRACE = res.instructions_and_trace
    if res.exec_time_ns is not None:
        print(f"[kernel] HW exec {res.exec_time_ns} ns", file=sys.stderr, flush=True)

    outs = []
    for i in range(NCORES):
        o = res.results[i]["out"]  # [128, G, NJ, C]
        # out[g, B*c + 128*jg + p] = o[p, g, jg, c]
        full = o.transpose(1, 3, 2, 0).reshape(G, C * B)
        outs.append(full[:, :LAGS])
    return np.concatenate(outs, axis=0).reshape(NB_PAIRS, NCH, LAGS)



# revision 23
# speedup vs baseline: 1.0141x; 1.0141x over previous
"""Trainium2 Bass kernel: batched time-domain cross-correlation.

Computes, for each of 2048 (=64x32) independent pairs (fp32):
    out[g, l] = sum_k d1[g, k + l - 301] * d2[g, k],   l in [0, 603)

Algorithm: overlap-save block correlation in a half-shift (negacyclic)
real-DFT basis, so every matmul has a *shared* stationary operand (the
transform matrices) and batches all pairs in the moving operand:

  xp = d1 zero-padded/shifted; y = d2 zero-padded.
  out[B*c + j] = sum_v corr(w_{v+c}, y_v)[j]     (j in [0, B))
    w_s = xp[B*s : B*s + 2B]  (windows, stride B, length N=2B)
    y_v = y[B*v : B*v + B]    (blocks, zero-padded to N)
  Per-block circular corr via length-N negacyclic real DFT:
    bins k: Ur[k] = sum_n u[n] cos(pi n (2k+1)/N)
            Ui[k] = -sum_n u[n] sin(pi n (2k+1)/N),  k in [0, B)
    Z = X * conj(Y):  Zr = XrYr + XiYi ; Zi = XiYr - XrYi
    z[0:B] = Minv @ [Zr; Zi]  (exact: aliasing only corrupts j > B)

Mapping: forward transforms + inverse are PE matmuls with shared
stationaries; the pointwise spectral products run on the Vector engine
with the v-sum done by segmented tensor_reduce.

Sharding: data-parallel over the 2048 pairs, 256 pairs per core, 8 cores.
"""

import math
import os
import sys

import ml_dtypes
import numpy as np

if "/opt/trn_rl_repo" not in sys.path:  # harness safety; axon site usually set
    sys.path.insert(0, "/opt/trn_rl_repo")

import concourse.bacc as bacc
import concourse.bass as bass
import concourse.mybir as mybir
import concourse.tile as tile
from concourse.bass_utils import run_bass_kernel_spmd

# ---- problem constants (hardcoded per contest contract) ----
NB_PAIRS, NCH, NT = 64, 32, 3000
LAGS = 603
SHIFT = 301  # NLAG + 1
NCORES = 8
G = (NB_PAIRS * NCH) // NCORES  # 256 pairs per core

# ---- tunables ----
B = int(os.environ.get("KB", "384"))  # lag/block granularity; N = 2B
GH = int(os.environ.get("KGH", "64"))  # pairs per g-chunk (SBUF working set)
# of every RED_FRAC product/tree ops, GP_FRAC go to GpSimd and the rest to DVE
GP_FRAC = int(os.environ.get("KGP", "1"))
RED_FRAC = int(os.environ.get("KRF", "4"))
DT_MM = mybir.dt.bfloat16  # matmul moving/stationary dtype
DT_Z = mybir.dt.bfloat16  # spectra / reduced-product dtype
DT_VE = mybir.dt.bfloat16  # elementwise product dtype (2x DVE rate)
NP_MM = ml_dtypes.bfloat16

# derived
N = 2 * B
V = math.ceil(NT / B)  # y blocks
C = math.ceil(LAGS / B)  # output lag blocks
S = V + C - 1  # x windows
SP = S  # no padding needed for bf16 matmuls
CP = C
BS = B // 128  # 128-chunks per B
NQ = N // 128  # contraction chunks of a full window
NJ = B // 128  # 128-chunks of B (bins halves / out j groups)
NR = 2 * NJ  # psum bin groups of the spectrum
U = (SP - 1) * BS + NQ  # 128-chunks in xp (covers padded windows)
NBB = U * 128
W = (V * B) // 128  # 128-chunks in y
# uneven pair-chunks sized so each chunk's x-fwd psum group fits one bank
# (gh*SP <= 512 free fp32) -> one matmul group per (r, chunk): fewest PE instrs
GHX = 512 // SP  # 56 for SP=9
_chunks = []
_g = 0
while _g < G:
    _chunks.append((_g, min(GHX, G - _g)))
    _g += min(GHX, G - _g)
GHMAX = max(gh for _, gh in _chunks)
# inverse groups aligned to chunk boundaries (2 groups, ~half the pairs each)
_half = _chunks[len(_chunks) // 2][0]
_IGROUPS = [(0, _half), (_half, G - _half)]

_PE_CACHE = {}
LAST_EXEC_NS = None
LAST_TRACE = None


def _matrices():
    n = np.arange(N, dtype=np.float64)[:, None]
    k = np.arange(B, dtype=np.float64)[None, :]
    theta = np.pi * n * (2 * k + 1) / N
    ffull = np.concatenate([np.cos(theta), -np.sin(theta)], axis=1)  # [N, 2B]
    minv = np.linalg.inv(ffull.T)[:B, :]  # [B, 2B]
    return ffull.astype(np.float32), minv.astype(np.float32)


def _const_tiles():
    """FW [128, NR*NQ*128]: FW[i, ((r*NQ)+q)*128 + col] = Ffull[128q+i, 128r+col]
    (r-major so each r's blocks are one contiguous DMA piece)
    MT [128, 3*NJ*NJ*128]: for zg in {Mr, Mi, -Mi}:
        MT[i, ((zg*NJ + rh)*NJ + jg)*128 + col] = M[128jg + col, 128rh + i]
    """
    ffull, minv = _matrices()
    fw = np.zeros((128, NR * NQ * 128), dtype=np.float32)
    for q in range(NQ):
        for r in range(NR):
            fw[:, (r * NQ + q) * 128 : (r * NQ + q + 1) * 128] = ffull[
                128 * q : 128 * (q + 1), 128 * r : 128 * (r + 1)
            ]
    mr = minv[:, :B]
    mi = minv[:, B:]
    mats = [mr, mi, -mi]
    mt = np.zeros((128, 3 * NJ * NJ * 128), dtype=np.float32)
    for zg in range(3):
        for rh in range(NJ):
            for jg in range(NJ):
                blk = mats[zg][128 * jg : 128 * (jg + 1), 128 * rh : 128 * (rh + 1)]
                base = ((zg * NJ + rh) * NJ + jg) * 128
                mt[:, base : base + 128] = blk.T
    return fw, mt


def build_kernel():
    nc = bacc.Bacc(
        "TRN2",
        target_bir_lowering=False,
        debug=False,
        num_devices=NCORES,
    )

    xp_d = nc.dram_tensor("xp", [128, G, U], DT_MM, kind="ExternalInput")
    yp_d = nc.dram_tensor("yp", [128, G, W], DT_MM, kind="ExternalInput")
    fw_d = nc.dram_tensor("fw", [128, NR * NQ * 128], DT_MM, kind="ExternalInput")
    mt_d = nc.dram_tensor("mt", [128, 3 * NJ * NJ * 128], DT_Z, kind="ExternalInput")
    out_d = nc.dram_tensor("out", [128, G, NJ, C], mybir.dt.float32,
                           kind="ExternalOutput")

    with tile.TileContext(nc, trace_sim=False) as tc:
        with (
            tc.tile_pool(name="const", bufs=1) as cpool,
            tc.tile_pool(name="io", bufs=2) as iopool,
            tc.tile_pool(name="spec", bufs=2) as spool,
            tc.tile_pool(name="work", bufs=3) as wpool,
            tc.tile_pool(name="zpool", bufs=1) as zpool,
            tc.tile_pool(name="psum", bufs=1, space=bass.MemorySpace.PSUM) as ppool,
        ):
            fw_t = cpool.tile([128, NR * NQ * 128], DT_MM, tag="fw")
            mt_t = cpool.tile([128, 3 * NJ * NJ * 128], DT_Z, tag="mt")
            zr = zpool.tile([128, NJ, G, CP], DT_Z, tag="zr")
            zi = zpool.tile([128, NJ, G, 2, CP], DT_Z, tag="zi")
            if CP > C:
                nc.gpsimd.memset(zr[:, :, :, C:], 0.0)
                nc.gpsimd.memset(zi[:, :, :, :, C:], 0.0)

            tt_i = 0

            def tt_eng(gpf, rdf):
                # weighted DVE/GpSimd split over all product/add ops:
                # gpf of every rdf ops go to GpSimd, rest to DVE
                nonlocal tt_i
                tt_i += 1
                if rdf and (tt_i - 1) % rdf < gpf:
                    return nc.gpsimd
                return nc.vector

            outt = iopool.tile([128, G, NJ, C], mybir.dt.float32, tag="outt", bufs=1)

            def emit_inverse(fgi):
                ig0, ign = _IGROUPS[fgi]
                gsl = slice(ig0, ig0 + ign)
                for jg in range(NJ):
                    ps = ppool.tile([128, GHMAX * 3, CP], mybir.dt.float32,
                                    tag="psC", bufs=2)
                    ps = ps[:, :ign, :]
                    nmm = 3 * NJ
                    i = 0
                    for rh in range(NJ):
                        srcs = (
                            (0, zr[:, rh, gsl, :]),
                            (1, zi[:, rh, gsl, 0, :]),
                            (2, zi[:, rh, gsl, 1, :]),
                        )
                        for zg, rhs in srcs:
                            lhsT = mt_t[
                                :,
                                ((zg * NJ + rh) * NJ + jg) * 128 :
                                ((zg * NJ + rh) * NJ + jg + 1) * 128,
                            ]
                            nc.tensor.matmul(
                                ps[:], lhsT, rhs,
                                start=(i == 0), stop=(i == nmm - 1),
                            )
                            i += 1
                    nc.scalar.copy(out=outt[:, gsl, jg, :], in_=ps[:, :, :C])
                nc.sync.dma_start(
                    out_d.ap()[:, gsl, :, :], outt[:, gsl, :, :]
                )

            inv_emitted = 0
            for ci, (g0, gh) in enumerate(_chunks):
                last = ci == len(_chunks) - 1
                xin = iopool.tile([128, GHMAX, U], DT_MM, tag="xin", bufs=3)
                yin = iopool.tile([128, GHMAX, W], DT_MM, tag="yin", bufs=3)
                nc.sync.dma_start(xin[:, :gh, :], xp_d.ap()[:, g0 : g0 + gh, :])
                nc.sync.dma_start(yin[:, :gh, :], yp_d.ap()[:, g0 : g0 + gh, :])
                if ci == 1:
                    # mt is first needed by the first deferred inverse
                    nc.sync.dma_start(mt_t[:], mt_d.ap())
                if ci == 0:
                    # consts after the first input tiles: r-pieces in use order
                    r_order0 = [x for rh in range(NJ) for x in (rh, NJ + rh)]
                    for r in r_order0:
                        nc.sync.dma_start(
                            fw_t[:, r * NQ * 128 : (r + 1) * NQ * 128],
                            fw_d.ap()[:, r * NQ * 128 : (r + 1) * NQ * 128],
                        )

                xs = spool.tile([128, NR, GHMAX, SP], DT_VE, tag="xs")
                ys = spool.tile([128, NR, GHMAX, V], DT_VE, tag="ys")

                # ---- forward transforms, x and y interleaved per bin
                # group; r-order pairs (rh, NJ+rh) so PW group rh unblocks
                # after two r-iterations
                r_order = [x for rh in range(NJ) for x in (rh, NJ + rh)]
                for r in r_order:
                    ps = ppool.tile([128, GHMAX, SP], mybir.dt.float32, tag="psA",
                                    bufs=4)
                    for q in range(NQ):
                        lhsT = fw_t[:, (r * NQ + q) * 128 : (r * NQ + q + 1) * 128]
                        rhs = xin[:, 0:gh, q : q + BS * (SP - 1) + 1 : BS]
                        nc.tensor.matmul(
                            ps[:, :gh, :], lhsT, rhs,
                            start=(q == 0), stop=(q == NQ - 1),
                        )
                    nc.scalar.copy(out=xs[:, r, 0:gh, :], in_=ps[:, :gh, :])
                    ps = ppool.tile([128, GHMAX, V], mybir.dt.float32, tag="psB",
                                    bufs=2)
                    for q in range(NJ):
                        lhsT = fw_t[:, (r * NQ + q) * 128 : (r * NQ + q + 1) * 128]
                        rhs = yin[:, 0:gh, q : q + BS * (V - 1) + 1 : BS]
                        nc.tensor.matmul(
                            ps[:, :gh, :], lhsT, rhs,
                            start=(q == 0), stop=(q == NJ - 1),
                        )
                    nc.scalar.copy(out=ys[:, r, 0:gh, :], in_=ps[:, :gh, :])

                # deferred inverse: emit groups whose products finished in
                # prior chunks AFTER this chunk's forward matmuls, so the PE
                # queue never stalls waiting on the product engines
                while (
                    inv_emitted < len(_IGROUPS)
                    and _IGROUPS[inv_emitted][0] + _IGROUPS[inv_emitted][1] <= g0
                ):
                    emit_inverse(inv_emitted)
                    inv_emitted += 1

                # ---- pointwise products + v-sum tree (DVE + GpSimd) ----
                # bias the last chunk toward DVE (faster) to shrink the tail
                gpf, rdf = (1, 8) if last else (GP_FRAC, RED_FRAC)
                HV = V // 2
                for c in range(C):
                    for rh in range(NJ):
                        with nc.allow_low_precision("bf16 spectra products"):
                            pr = wpool.tile([128, GHMAX, 2, V], DT_VE, tag="pr",
                                            bufs=4)
                            tt_eng(gpf, rdf).tensor_mul(
                                pr[:, :gh, 0, :],
                                xs[:, rh, :gh, c : c + V],
                                ys[:, rh, :gh, :],
                            )
                            tt_eng(gpf, rdf).tensor_mul(
                                pr[:, :gh, 1, :],
                                xs[:, NJ + rh, :gh, c : c + V],
                                ys[:, NJ + rh, :gh, :],
                            )
                            # tree-sum over (2, V): stride-1 halves each pass
                            w4 = wpool.tile([128, GHMAX, 2, HV], DT_VE, tag="w4",
                                            bufs=4)
                            tt_eng(gpf, rdf).tensor_add(
                                w4[:, :gh], pr[:, :gh, :, 0:HV],
                                pr[:, :gh, :, HV:V],
                            )
                            w2 = wpool.tile([128, GHMAX, 2, HV // 2], DT_VE,
                                            tag="w2", bufs=4)
                            tt_eng(gpf, rdf).tensor_add(
                                w2[:, :gh], w4[:, :gh, :, 0 : HV // 2],
                                w4[:, :gh, :, HV // 2 : HV],
                            )
                            w1 = wpool.tile([128, GHMAX, 2], DT_VE, tag="w1",
                                            bufs=4)
                            tt_eng(gpf, rdf).tensor_add(
                                w1[:, :gh], w2[:, :gh, :, 0], w2[:, :gh, :, 1]
                            )
                            tt_eng(gpf, rdf).tensor_add(
                                zr[:, rh, g0 : g0 + gh, c], w1[:, :gh, 0],
                                w1[:, :gh, 1],
                            )
                            pr2 = wpool.tile([128, GHMAX, 2, V], DT_VE, tag="pr",
                                             bufs=4)
                            tt_eng(gpf, rdf).tensor_mul(
                                pr2[:, :gh, 0, :],
                                xs[:, NJ + rh, :gh, c : c + V],
                                ys[:, rh, :gh, :],
                            )
                            tt_eng(gpf, rdf).tensor_mul(
                                pr2[:, :gh, 1, :],
                                xs[:, rh, :gh, c : c + V],
                                ys[:, NJ + rh, :gh, :],
                            )
                            m4 = wpool.tile([128, GHMAX, 2, HV], DT_VE, tag="w4",
                                            bufs=4)
                            tt_eng(gpf, rdf).tensor_add(
                                m4[:, :gh], pr2[:, :gh, :, 0:HV],
                                pr2[:, :gh, :, HV:V],
                            )
                            m2 = wpool.tile([128, GHMAX, 2, HV // 2], DT_VE,
                                            tag="w2", bufs=4)
                            tt_eng(gpf, rdf).tensor_add(
                                m2[:, :gh], m4[:, :gh, :, 0 : HV // 2],
                                m4[:, :gh, :, HV // 2 : HV],
                            )
                            tt_eng(gpf, rdf).tensor_add(
                                zi[:, rh, g0 : g0 + gh, :, c], m2[:, :gh, :, 0],
                                m2[:, :gh, :, 1],
                            )

            while inv_emitted < len(_IGROUPS):
                emit_inverse(inv_emitted)
                inv_emitted += 1

    nc.compile()
    return nc


def _prep_core_inputs(d1f, d2f, fw, mt, core):
    """d1f/d2f: [2048, 3000] fp32. Returns the in_map for `core`."""
    sl = slice(core * G, (core + 1) * G)
    x = d1f[sl]
    y = d2f[sl]
    xp = np.zeros((G, NBB), dtype=np.float32)
    xp[:, SHIFT : SHIFT + NT] = x
    yp = np.zeros((G, V * B), dtype=np.float32)
    yp[:, :NT] = y
    # device layouts: xpT[p, g, u] = xp[g, 128u + p]
    xpT = np.ascontiguousarray(xp.reshape(G, U, 128).transpose(2, 0, 1)).astype(NP_MM)
    ypT = np.ascontiguousarray(yp.reshape(G, W, 128).transpose(2, 0, 1)).astype(NP_MM)
    return {"xp": xpT, "yp": ypT, "fw": fw.astype(NP_MM), "mt": mt.astype(NP_MM)}


def kernel(data1: np.ndarray, data2: np.ndarray) -> np.ndarray:
    import time

    d1f = np.ascontiguousarray(data1, dtype=np.float32).reshape(-1, NT)
    d2f = np.ascontiguousarray(data2, dtype=np.float32).reshape(-1, NT)
    fw, mt = _const_tiles()

    t0 = time.time()
    if "nc" not in _PE_CACHE:
        _PE_CACHE["nc"] = build_kernel()
    nc = _PE_CACHE["nc"]
    print(f"[kernel] build+compile {time.time() - t0:.1f}s", file=sys.stderr,
          flush=True)

    in_maps = [_prep_core_inputs(d1f, d2f, fw, mt, i) for i in range(NCORES)]
    t0 = time.time()
    res = run_bass_kernel_spmd(nc, in_maps, core_ids=list(range(NCORES)))
    print(f"[kernel] spmd run {time.time() - t0:.1f}s", file=sys.stderr, flush=True)
    global LAST_EXEC_NS, LAST_TRACE
    LAST_EXEC_NS = res.exec_time_ns
    LAST_TRACE = res.instructions_and_trace
    if res.exec_time_ns is not None:
        print(f"[kernel] HW exec {res.exec_time_ns} ns", file=sys.stderr, flush=True)

    outs = []
    for i in range(NCORES):
        o = res.results[i]["out"]  # [128, G, NJ, C]
        # out[g, B*c + 128*jg + p] = o[p, g, jg, c]
        full = o.transpose(1, 3, 2, 0).reshape(G, C * B)
        outs.append(full[:, :LAGS])
    return np.concatenate(outs, axis=0).reshape(NB_PAIRS, NCH, LAGS)



# revision 25
# speedup vs baseline: 1.0468x; 1.0323x over previous
"""Trainium2 Bass kernel: batched time-domain cross-correlation.

Computes, for each of 2048 (=64x32) independent pairs (fp32):
    out[g, l] = sum_k d1[g, k + l - 301] * d2[g, k],   l in [0, 603)

Algorithm: overlap-save block correlation in a half-shift (negacyclic)
real-DFT basis, so every matmul has a *shared* stationary operand (the
transform matrices) and batches all pairs in the moving operand:

  xp = d1 zero-padded/shifted; y = d2 zero-padded.
  out[B*c + j] = sum_v corr(w_{v+c}, y_v)[j]     (j in [0, B))
    w_s = xp[B*s : B*s + 2B]  (windows, stride B, length N=2B)
    y_v = y[B*v : B*v + B]    (blocks, zero-padded to N)
  Per-block circular corr via length-N negacyclic real DFT:
    bins k: Ur[k] = sum_n u[n] cos(pi n (2k+1)/N)
            Ui[k] = -sum_n u[n] sin(pi n (2k+1)/N),  k in [0, B)
    Z = X * conj(Y):  Zr = XrYr + XiYi ; Zi = XiYr - XrYi
    z[0:B] = Minv @ [Zr; Zi]  (exact: aliasing only corrupts j > B)

Mapping: forward transforms + inverse are PE matmuls with shared
stationaries; the pointwise spectral products run on the Vector engine
with the v-sum done by segmented tensor_reduce.

Sharding: data-parallel over the 2048 pairs, 256 pairs per core, 8 cores.
"""

import math
import os
import sys

import ml_dtypes
import numpy as np

if "/opt/trn_rl_repo" not in sys.path:  # harness safety; axon site usually set
    sys.path.insert(0, "/opt/trn_rl_repo")

import concourse.bacc as bacc
import concourse.bass as bass
import concourse.mybir as mybir
import concourse.tile as tile
from concourse.bass_utils import run_bass_kernel_spmd

# ---- problem constants (hardcoded per contest contract) ----
NB_PAIRS, NCH, NT = 64, 32, 3000
LAGS = 603
SHIFT = 301  # NLAG + 1
NCORES = 8
G = (NB_PAIRS * NCH) // NCORES  # 256 pairs per core

# ---- tunables ----
B = int(os.environ.get("KB", "384"))  # lag/block granularity; N = 2B
GH = int(os.environ.get("KGH", "64"))  # pairs per g-chunk (SBUF working set)
# of every RED_FRAC product/tree ops, GP_FRAC go to GpSimd and the rest to DVE
GP_FRAC = int(os.environ.get("KGP", "1"))
RED_FRAC = int(os.environ.get("KRF", "4"))
DT_MM = mybir.dt.bfloat16  # matmul moving/stationary dtype
DT_Z = mybir.dt.bfloat16  # spectra / reduced-product dtype
DT_VE = mybir.dt.bfloat16  # elementwise product dtype (2x DVE rate)
NP_MM = ml_dtypes.bfloat16

# derived
N = 2 * B
V = math.ceil(NT / B)  # y blocks
C = math.ceil(LAGS / B)  # output lag blocks
S = V + C - 1  # x windows
SP = S  # no padding needed for bf16 matmuls
CP = C
BS = B // 128  # 128-chunks per B
NQ = N // 128  # contraction chunks of a full window
NJ = B // 128  # 128-chunks of B (bins halves / out j groups)
NR = 2 * NJ  # psum bin groups of the spectrum
U = (SP - 1) * BS + NQ  # 128-chunks in xp (covers padded windows)
NBB = U * 128
W = (V * B) // 128  # 128-chunks in y
# uneven pair-chunks sized so each chunk's x-fwd psum group fits one bank
# (gh*SP <= 512 free fp32) -> one matmul group per (r, chunk): fewest PE instrs
GHX = 512 // SP  # 56 for SP=9
_chunks = []
_g = 0
while _g < G:
    _chunks.append((_g, min(GHX, G - _g)))
    _g += min(GHX, G - _g)
GHMAX = max(gh for _, gh in _chunks)
# inverse groups aligned to chunk boundaries; small final group = short tail
_b1 = _chunks[2][0]
_b2 = _chunks[4][0]
_IGROUPS = [(0, _b1), (_b1, _b2 - _b1), (_b2, G - _b2)]

_PE_CACHE = {}
LAST_EXEC_NS = None
LAST_TRACE = None


def _matrices():
    n = np.arange(N, dtype=np.float64)[:, None]
    k = np.arange(B, dtype=np.float64)[None, :]
    theta = np.pi * n * (2 * k + 1) / N
    ffull = np.concatenate([np.cos(theta), -np.sin(theta)], axis=1)  # [N, 2B]
    minv = np.linalg.inv(ffull.T)[:B, :]  # [B, 2B]
    return ffull.astype(np.float32), minv.astype(np.float32)


def _const_tiles():
    """FW [128, NR*NQ*128]: FW[i, ((r*NQ)+q)*128 + col] = Ffull[128q+i, 128r+col]
    (r-major so each r's blocks are one contiguous DMA piece)
    MT [128, 3*NJ*NJ*128]: for zg in {Mr, Mi, -Mi}:
        MT[i, ((zg*NJ + rh)*NJ + jg)*128 + col] = M[128jg + col, 128rh + i]
    """
    ffull, minv = _matrices()
    fw = np.zeros((128, NR * NQ * 128), dtype=np.float32)
    for q in range(NQ):
        for r in range(NR):
            fw[:, (r * NQ + q) * 128 : (r * NQ + q + 1) * 128] = ffull[
                128 * q : 128 * (q + 1), 128 * r : 128 * (r + 1)
            ]
    mr = minv[:, :B]
    mi = minv[:, B:]
    mats = [mr, mi, -mi]
    mt = np.zeros((128, 3 * NJ * NJ * 128), dtype=np.float32)
    for zg in range(3):
        for rh in range(NJ):
            for jg in range(NJ):
                blk = mats[zg][128 * jg : 128 * (jg + 1), 128 * rh : 128 * (rh + 1)]
                base = ((zg * NJ + rh) * NJ + jg) * 128
                mt[:, base : base + 128] = blk.T
    return fw, mt


def build_kernel():
    nc = bacc.Bacc(
        "TRN2",
        target_bir_lowering=False,
        debug=False,
        num_devices=NCORES,
    )

    xp_d = nc.dram_tensor("xp", [128, G, U], DT_MM, kind="ExternalInput")
    yp_d = nc.dram_tensor("yp", [128, G, W], DT_MM, kind="ExternalInput")
    fw_d = nc.dram_tensor("fw", [128, NR * NQ * 128], DT_MM, kind="ExternalInput")
    mt_d = nc.dram_tensor("mt", [128, 3 * NJ * NJ * 128], DT_Z, kind="ExternalInput")
    out_d = nc.dram_tensor("out", [128, G, NJ, C], mybir.dt.float32,
                           kind="ExternalOutput")

    with tile.TileContext(nc, trace_sim=False) as tc:
        with (
            tc.tile_pool(name="const", bufs=1) as cpool,
            tc.tile_pool(name="io", bufs=2) as iopool,
            tc.tile_pool(name="spec", bufs=2) as spool,
            tc.tile_pool(name="work", bufs=3) as wpool,
            tc.tile_pool(name="zpool", bufs=1) as zpool,
            tc.tile_pool(name="psum", bufs=1, space=bass.MemorySpace.PSUM) as ppool,
        ):
            fw_t = cpool.tile([128, NR * NQ * 128], DT_MM, tag="fw")
            mt_t = cpool.tile([128, 3 * NJ * NJ * 128], DT_Z, tag="mt")
            zr = zpool.tile([128, NJ, G, CP], DT_Z, tag="zr")
            zi = zpool.tile([128, NJ, G, 2, CP], DT_Z, tag="zi")
            if CP > C:
                nc.gpsimd.memset(zr[:, :, :, C:], 0.0)
                nc.gpsimd.memset(zi[:, :, :, :, C:], 0.0)

            tt_i = 0

            def tt_eng(gpf, rdf):
                # weighted DVE/GpSimd split over all product/add ops:
                # gpf of every rdf ops go to GpSimd, rest to DVE
                nonlocal tt_i
                tt_i += 1
                if rdf and (tt_i - 1) % rdf < gpf:
                    return nc.gpsimd
                return nc.vector

            outt = iopool.tile([128, G, NJ, C], mybir.dt.float32, tag="outt", bufs=1)

            def emit_inverse(fgi):
                ig0, ign = _IGROUPS[fgi]
                gsl = slice(ig0, ig0 + ign)
                for jg in range(NJ):
                    ps = ppool.tile([128, GHMAX * 3, CP], mybir.dt.float32,
                                    tag="psC", bufs=2)
                    ps = ps[:, :ign, :]
                    nmm = 3 * NJ
                    i = 0
                    for rh in range(NJ):
                        srcs = (
                            (0, zr[:, rh, gsl, :]),
                            (1, zi[:, rh, gsl, 0, :]),
                            (2, zi[:, rh, gsl, 1, :]),
                        )
                        for zg, rhs in srcs:
                            lhsT = mt_t[
                                :,
                                ((zg * NJ + rh) * NJ + jg) * 128 :
                                ((zg * NJ + rh) * NJ + jg + 1) * 128,
                            ]
                            nc.tensor.matmul(
                                ps[:], lhsT, rhs,
                                start=(i == 0), stop=(i == nmm - 1),
                            )
                            i += 1
                    nc.scalar.copy(out=outt[:, gsl, jg, :], in_=ps[:, :, :C])
                nc.sync.dma_start(
                    out_d.ap()[:, gsl, :, :], outt[:, gsl, :, :]
                )

            inv_emitted = 0
            for ci, (g0, gh) in enumerate(_chunks):
                last = ci >= len(_chunks) - 2
                xin = iopool.tile([128, GHMAX, U], DT_MM, tag="xin", bufs=3)
                yin = iopool.tile([128, GHMAX, W], DT_MM, tag="yin", bufs=3)
                nc.sync.dma_start(xin[:, :gh, :], xp_d.ap()[:, g0 : g0 + gh, :])
                nc.sync.dma_start(yin[:, :gh, :], yp_d.ap()[:, g0 : g0 + gh, :])
                if ci == 1:
                    # mt is first needed by the first deferred inverse
                    nc.sync.dma_start(mt_t[:], mt_d.ap())
                if ci == 0:
                    # consts after the first input tiles: r-pieces in use order
                    r_order0 = [x for rh in range(NJ) for x in (rh, NJ + rh)]
                    for r in r_order0:
                        nc.sync.dma_start(
                            fw_t[:, r * NQ * 128 : (r + 1) * NQ * 128],
                            fw_d.ap()[:, r * NQ * 128 : (r + 1) * NQ * 128],
                        )

                xs = spool.tile([128, NR, GHMAX, SP], DT_VE, tag="xs")
                ys = spool.tile([128, NR, GHMAX, V], DT_VE, tag="ys")

                # ---- forward transforms, x and y interleaved per bin
                # group; r-order pairs (rh, NJ+rh) so PW group rh unblocks
                # after two r-iterations
                r_order = [x for rh in range(NJ) for x in (rh, NJ + rh)]
                for r in r_order:
                    ps = ppool.tile([128, GHMAX, SP], mybir.dt.float32, tag="psA",
                                    bufs=4)
                    for q in range(NQ):
                        lhsT = fw_t[:, (r * NQ + q) * 128 : (r * NQ + q + 1) * 128]
                        rhs = xin[:, 0:gh, q : q + BS * (SP - 1) + 1 : BS]
                        nc.tensor.matmul(
                            ps[:, :gh, :], lhsT, rhs,
                            start=(q == 0), stop=(q == NQ - 1),
                        )
                    nc.scalar.copy(out=xs[:, r, 0:gh, :], in_=ps[:, :gh, :])
                    ps = ppool.tile([128, GHMAX, V], mybir.dt.float32, tag="psB",
                                    bufs=2)
                    for q in range(NJ):
                        lhsT = fw_t[:, (r * NQ + q) * 128 : (r * NQ + q + 1) * 128]
                        rhs = yin[:, 0:gh, q : q + BS * (V - 1) + 1 : BS]
                        nc.tensor.matmul(
                            ps[:, :gh, :], lhsT, rhs,
                            start=(q == 0), stop=(q == NJ - 1),
                        )
                    nc.scalar.copy(out=ys[:, r, 0:gh, :], in_=ps[:, :gh, :])

                # deferred inverse: emit groups whose products finished in
                # prior chunks AFTER this chunk's forward matmuls, so the PE
                # queue never stalls waiting on the product engines
                while (
                    inv_emitted < len(_IGROUPS)
                    and _IGROUPS[inv_emitted][0] + _IGROUPS[inv_emitted][1] <= g0
                ):
                    emit_inverse(inv_emitted)
                    inv_emitted += 1

                # ---- pointwise products + v-sum tree (DVE + GpSimd) ----
                # bias the last chunk toward DVE (faster) to shrink the tail
                gpf, rdf = (1, 8) if last else (GP_FRAC, RED_FRAC)
                HV = V // 2
                for c in range(C):
                    for rh in range(NJ):
                        with nc.allow_low_precision("bf16 spectra products"):
                            pr = wpool.tile([128, GHMAX, 2, V], DT_VE, tag="pr",
                                            bufs=4)
                            tt_eng(gpf, rdf).tensor_mul(
                                pr[:, :gh, 0, :],
                                xs[:, rh, :gh, c : c + V],
                                ys[:, rh, :gh, :],
                            )
                            tt_eng(gpf, rdf).tensor_mul(
                                pr[:, :gh, 1, :],
                                xs[:, NJ + rh, :gh, c : c + V],
                                ys[:, NJ + rh, :gh, :],
                            )
                            # tree-sum over (2, V): stride-1 halves each pass
                            w4 = wpool.tile([128, GHMAX, 2, HV], DT_VE, tag="w4",
                                            bufs=4)
                            tt_eng(gpf, rdf).tensor_add(
                                w4[:, :gh], pr[:, :gh, :, 0:HV],
                                pr[:, :gh, :, HV:V],
                            )
                            w2 = wpool.tile([128, GHMAX, 2, HV // 2], DT_VE,
                                            tag="w2", bufs=4)
                            tt_eng(gpf, rdf).tensor_add(
                                w2[:, :gh], w4[:, :gh, :, 0 : HV // 2],
                                w4[:, :gh, :, HV // 2 : HV],
                            )
                            w1 = wpool.tile([128, GHMAX, 2], DT_VE, tag="w1",
                                            bufs=4)
                            tt_eng(gpf, rdf).tensor_add(
                                w1[:, :gh], w2[:, :gh, :, 0], w2[:, :gh, :, 1]
                            )
                            tt_eng(gpf, rdf).tensor_add(
                                zr[:, rh, g0 : g0 + gh, c], w1[:, :gh, 0],
                                w1[:, :gh, 1],
                            )
                            pr2 = wpool.tile([128, GHMAX, 2, V], DT_VE, tag="pr",
                                             bufs=4)
                            tt_eng(gpf, rdf).tensor_mul(
                                pr2[:, :gh, 0, :],
                                xs[:, NJ + rh, :gh, c : c + V],
                                ys[:, rh, :gh, :],
                            )
                            tt_eng(gpf, rdf).tensor_mul(
                                pr2[:, :gh, 1, :],
                                xs[:, rh, :gh, c : c + V],
                                ys[:, NJ + rh, :gh, :],
                            )
                            m4 = wpool.tile([128, GHMAX, 2, HV], DT_VE, tag="w4",
                                            bufs=4)
                            tt_eng(gpf, rdf).tensor_add(
                                m4[:, :gh], pr2[:, :gh, :, 0:HV],
                                pr2[:, :gh, :, HV:V],
                            )
                            m2 = wpool.tile([128, GHMAX, 2, HV // 2], DT_VE,
                                            tag="w2", bufs=4)
                            tt_eng(gpf, rdf).tensor_add(
                                m2[:, :gh], m4[:, :gh, :, 0 : HV // 2],
                                m4[:, :gh, :, HV // 2 : HV],
                            )
                            tt_eng(gpf, rdf).tensor_add(
                                zi[:, rh, g0 : g0 + gh, :, c], m2[:, :gh, :, 0],
                                m2[:, :gh, :, 1],
                            )

            while inv_emitted < len(_IGROUPS):
                emit_inverse(inv_emitted)
                inv_emitted += 1

    nc.compile()
    return nc


def _prep_core_inputs(d1f, d2f, fw, mt, core):
    """d1f/d2f: [2048, 3000] fp32. Returns the in_map for `core`."""
    sl = slice(core * G, (core + 1) * G)
    x = d1f[sl]
    y = d2f[sl]
    xp = np.zeros((G, NBB), dtype=np.float32)
    xp[:, SHIFT : SHIFT + NT] = x
    yp = np.zeros((G, V * B), dtype=np.float32)
    yp[:, :NT] = y
    # device layouts: xpT[p, g, u] = xp[g, 128u + p]
    xpT = np.ascontiguousarray(xp.reshape(G, U, 128).transpose(2, 0, 1)).astype(NP_MM)
    ypT = np.ascontiguousarray(yp.reshape(G, W, 128).transpose(2, 0, 1)).astype(NP_MM)
    return {"xp": xpT, "yp": ypT, "fw": fw.astype(NP_MM), "mt": mt.astype(NP_MM)}


def kernel(data1: np.ndarray, data2: np.ndarray) -> np.ndarray:
    import time

    d1f = np.ascontiguousarray(data1, dtype=np.float32).reshape(-1, NT)
    d2f = np.ascontiguousarray(data2, dtype=np.float32).reshape(-1, NT)
    fw, mt = _const_tiles()

    t0 = time.time()
    if "nc" not in _PE_CACHE:
        _PE_CACHE["nc"] = build_kernel()
    nc = _PE_CACHE["nc"]
    print(f"[kernel] build+compile {time.time() - t0:.1f}s", file=sys.stderr,
          flush=True)

    in_maps = [_prep_core_inputs(d1f, d2f, fw, mt, i) for i in range(NCORES)]
    t0 = time.time()
    res = run_bass_kernel_spmd(nc, in_maps, core_ids=list(range(NCORES)))
    print(f"[kernel] spmd run {time.time() - t0:.1f}s", file=sys.stderr, flush=True)
    global LAST_EXEC_NS, LAST_TRACE
    LAST_EXEC_NS = res.exec_time_ns
    LAST_TRACE = res.instructions_and_trace
    if res.exec_time_ns is not None:
        print(f"[kernel] HW exec {res.exec_time_ns} ns", file=sys.stderr, flush=True)

    outs = []
    for i in range(NCORES):
        o = res.results[i]["out"]  # [128, G, NJ, C]
        # out[g, B*c + 128*jg + p] = o[p, g, jg, c]
        full = o.transpose(1, 3, 2, 0).reshape(G, C * B)
        outs.append(full[:, :LAGS])
    return np.concatenate(outs, axis=0).reshape(NB_PAIRS, NCH, LAGS)



# revision 26
# speedup vs baseline: 1.0704x; 1.0225x over previous
"""Trainium2 Bass kernel: batched time-domain cross-correlation.

Computes, for each of 2048 (=64x32) independent pairs (fp32):
    out[g, l] = sum_k d1[g, k + l - 301] * d2[g, k],   l in [0, 603)

Algorithm: overlap-save block correlation in a half-shift (negacyclic)
real-DFT basis, so every matmul has a *shared* stationary operand (the
transform matrices) and batches all pairs in the moving operand:

  xp = d1 zero-padded/shifted; y = d2 zero-padded.
  out[B*c + j] = sum_v corr(w_{v+c}, y_v)[j]     (j in [0, B))
    w_s = xp[B*s : B*s + 2B]  (windows, stride B, length N=2B)
    y_v = y[B*v : B*v + B]    (blocks, zero-padded to N)
  Per-block circular corr via length-N negacyclic real DFT:
    bins k: Ur[k] = sum_n u[n] cos(pi n (2k+1)/N)
            Ui[k] = -sum_n u[n] sin(pi n (2k+1)/N),  k in [0, B)
    Z = X * conj(Y):  Zr = XrYr + XiYi ; Zi = XiYr - XrYi
    z[0:B] = Minv @ [Zr; Zi]  (exact: aliasing only corrupts j > B)

Mapping: forward transforms + inverse are PE matmuls with shared
stationaries; the pointwise spectral products run on the Vector engine
with the v-sum done by segmented tensor_reduce.

Sharding: data-parallel over the 2048 pairs, 256 pairs per core, 8 cores.
"""

import math
import os
import sys

import ml_dtypes
import numpy as np

if "/opt/trn_rl_repo" not in sys.path:  # harness safety; axon site usually set
    sys.path.insert(0, "/opt/trn_rl_repo")

import concourse.bacc as bacc
import concourse.bass as bass
import concourse.mybir as mybir
import concourse.tile as tile
from concourse.bass_utils import run_bass_kernel_spmd

# ---- problem constants (hardcoded per contest contract) ----
NB_PAIRS, NCH, NT = 64, 32, 3000
LAGS = 603
SHIFT = 301  # NLAG + 1
NCORES = 8
G = (NB_PAIRS * NCH) // NCORES  # 256 pairs per core

# ---- tunables ----
B = int(os.environ.get("KB", "384"))  # lag/block granularity; N = 2B
GH = int(os.environ.get("KGH", "64"))  # pairs per g-chunk (SBUF working set)
# of every RED_FRAC product/tree ops, GP_FRAC go to GpSimd and the rest to DVE
GP_FRAC = int(os.environ.get("KGP", "1"))
RED_FRAC = int(os.environ.get("KRF", "8"))
DT_MM = mybir.dt.bfloat16  # matmul moving/stationary dtype
DT_Z = mybir.dt.bfloat16  # spectra / reduced-product dtype
DT_VE = mybir.dt.bfloat16  # elementwise product dtype (2x DVE rate)
NP_MM = ml_dtypes.bfloat16

# derived
N = 2 * B
V = math.ceil(NT / B)  # y blocks
C = math.ceil(LAGS / B)  # output lag blocks
S = V + C - 1  # x windows
SP = S  # no padding needed for bf16 matmuls
CP = C
BS = B // 128  # 128-chunks per B
NQ = N // 128  # contraction chunks of a full window
NJ = B // 128  # 128-chunks of B (bins halves / out j groups)
NR = 2 * NJ  # psum bin groups of the spectrum
U = (SP - 1) * BS + NQ  # 128-chunks in xp (covers padded windows)
NBB = U * 128
W = (V * B) // 128  # 128-chunks in y
# uneven pair-chunks sized so each chunk's x-fwd psum group fits one bank
# (gh*SP <= 512 free fp32) -> one matmul group per (r, chunk): fewest PE instrs
GHX = 512 // SP  # 56 for SP=9
_chunks = []
_g = 0
while _g < G:
    _chunks.append((_g, min(GHX, G - _g)))
    _g += min(GHX, G - _g)
GHMAX = max(gh for _, gh in _chunks)
# inverse groups aligned to chunk boundaries; small final group = short tail
_b1 = _chunks[2][0]
_b2 = _chunks[4][0]
_IGROUPS = [(0, _b1), (_b1, _b2 - _b1), (_b2, G - _b2)]

_PE_CACHE = {}
LAST_EXEC_NS = None
LAST_TRACE = None


def _matrices():
    n = np.arange(N, dtype=np.float64)[:, None]
    k = np.arange(B, dtype=np.float64)[None, :]
    theta = np.pi * n * (2 * k + 1) / N
    ffull = np.concatenate([np.cos(theta), -np.sin(theta)], axis=1)  # [N, 2B]
    minv = np.linalg.inv(ffull.T)[:B, :]  # [B, 2B]
    return ffull.astype(np.float32), minv.astype(np.float32)


def _const_tiles():
    """FW [128, NR*NQ*128]: FW[i, ((r*NQ)+q)*128 + col] = Ffull[128q+i, 128r+col]
    (r-major so each r's blocks are one contiguous DMA piece)
    MT [128, 3*NJ*NJ*128]: for zg in {Mr, Mi, -Mi}:
        MT[i, ((zg*NJ + rh)*NJ + jg)*128 + col] = M[128jg + col, 128rh + i]
    """
    ffull, minv = _matrices()
    fw = np.zeros((128, NR * NQ * 128), dtype=np.float32)
    for q in range(NQ):
        for r in range(NR):
            fw[:, (r * NQ + q) * 128 : (r * NQ + q + 1) * 128] = ffull[
                128 * q : 128 * (q + 1), 128 * r : 128 * (r + 1)
            ]
    mr = minv[:, :B]
    mi = minv[:, B:]
    mats = [mr, mi, -mi]
    mt = np.zeros((128, 3 * NJ * NJ * 128), dtype=np.float32)
    for zg in range(3):
        for rh in range(NJ):
            for jg in range(NJ):
                blk = mats[zg][128 * jg : 128 * (jg + 1), 128 * rh : 128 * (rh + 1)]
                base = ((zg * NJ + rh) * NJ + jg) * 128
                mt[:, base : base + 128] = blk.T
    return fw, mt


def build_kernel():
    nc = bacc.Bacc(
        "TRN2",
        target_bir_lowering=False,
        debug=False,
        num_devices=NCORES,
    )

    xp_d = nc.dram_tensor("xp", [128, G, U], DT_MM, kind="ExternalInput")
    yp_d = nc.dram_tensor("yp", [128, G, W], DT_MM, kind="ExternalInput")
    fw_d = nc.dram_tensor("fw", [128, NR * NQ * 128], DT_MM, kind="ExternalInput")
    mt_d = nc.dram_tensor("mt", [128, 3 * NJ * NJ * 128], DT_Z, kind="ExternalInput")
    out_d = nc.dram_tensor("out", [128, G, NJ, C], mybir.dt.float32,
                           kind="ExternalOutput")

    with tile.TileContext(nc, trace_sim=False) as tc:
        with (
            tc.tile_pool(name="const", bufs=1) as cpool,
            tc.tile_pool(name="io", bufs=2) as iopool,
            tc.tile_pool(name="spec", bufs=2) as spool,
            tc.tile_pool(name="work", bufs=3) as wpool,
            tc.tile_pool(name="zpool", bufs=1) as zpool,
            tc.tile_pool(name="psum", bufs=1, space=bass.MemorySpace.PSUM) as ppool,
        ):
            fw_t = cpool.tile([128, NR * NQ * 128], DT_MM, tag="fw")
            mt_t = cpool.tile([128, 3 * NJ * NJ * 128], DT_Z, tag="mt")
            zr = zpool.tile([128, NJ, G, CP], DT_Z, tag="zr")
            zi = zpool.tile([128, NJ, G, 2, CP], DT_Z, tag="zi")
            if CP > C:
                nc.gpsimd.memset(zr[:, :, :, C:], 0.0)
                nc.gpsimd.memset(zi[:, :, :, :, C:], 0.0)

            tt_i = 0

            def tt_eng(gpf, rdf):
                # weighted DVE/GpSimd split over all product/add ops:
                # gpf of every rdf ops go to GpSimd, rest to DVE
                nonlocal tt_i
                tt_i += 1
                if rdf and (tt_i - 1) % rdf < gpf:
                    return nc.gpsimd
                return nc.vector

            outt = iopool.tile([128, G, NJ, C], mybir.dt.float32, tag="outt", bufs=1)

            def emit_inverse(fgi):
                ig0, ign = _IGROUPS[fgi]
                gsl = slice(ig0, ig0 + ign)
                for jg in range(NJ):
                    ps = ppool.tile([128, GHMAX * 3, CP], mybir.dt.float32,
                                    tag="psC", bufs=2)
                    ps = ps[:, :ign, :]
                    nmm = 3 * NJ
                    i = 0
                    for rh in range(NJ):
                        srcs = (
                            (0, zr[:, rh, gsl, :]),
                            (1, zi[:, rh, gsl, 0, :]),
                            (2, zi[:, rh, gsl, 1, :]),
                        )
                        for zg, rhs in srcs:
                            lhsT = mt_t[
                                :,
                                ((zg * NJ + rh) * NJ + jg) * 128 :
                                ((zg * NJ + rh) * NJ + jg + 1) * 128,
                            ]
                            nc.tensor.matmul(
                                ps[:], lhsT, rhs,
                                start=(i == 0), stop=(i == nmm - 1),
                            )
                            i += 1
                    nc.scalar.copy(out=outt[:, gsl, jg, :], in_=ps[:, :, :C])
                nc.sync.dma_start(
                    out_d.ap()[:, gsl, :, :], outt[:, gsl, :, :]
                )

            inv_emitted = 0
            for ci, (g0, gh) in enumerate(_chunks):
                last = ci >= len(_chunks) - 2
                xin = iopool.tile([128, GHMAX, U], DT_MM, tag="xin", bufs=3)
                yin = iopool.tile([128, GHMAX, W], DT_MM, tag="yin", bufs=3)
                nc.sync.dma_start(xin[:, :gh, :], xp_d.ap()[:, g0 : g0 + gh, :])
                nc.sync.dma_start(yin[:, :gh, :], yp_d.ap()[:, g0 : g0 + gh, :])
                if ci == 1:
                    # mt is first needed by the first deferred inverse
                    nc.sync.dma_start(mt_t[:], mt_d.ap())
                if ci == 0:
                    # consts after the first input tiles: r-pieces in use order
                    r_order0 = [x for rh in range(NJ) for x in (rh, NJ + rh)]
                    for r in r_order0:
                        nc.sync.dma_start(
                            fw_t[:, r * NQ * 128 : (r + 1) * NQ * 128],
                            fw_d.ap()[:, r * NQ * 128 : (r + 1) * NQ * 128],
                        )

                xs = spool.tile([128, NR, GHMAX, SP], DT_VE, tag="xs")
                ys = spool.tile([128, NR, GHMAX, V], DT_VE, tag="ys")

                # ---- forward transforms, x and y interleaved per bin
                # group; r-order pairs (rh, NJ+rh) so PW group rh unblocks
                # after two r-iterations
                r_order = [x for rh in range(NJ) for x in (rh, NJ + rh)]
                for r in r_order:
                    ps = ppool.tile([128, GHMAX, SP], mybir.dt.float32, tag="psA",
                                    bufs=4)
                    for q in range(NQ):
                        lhsT = fw_t[:, (r * NQ + q) * 128 : (r * NQ + q + 1) * 128]
                        rhs = xin[:, 0:gh, q : q + BS * (SP - 1) + 1 : BS]
                        nc.tensor.matmul(
                            ps[:, :gh, :], lhsT, rhs,
                            start=(q == 0), stop=(q == NQ - 1),
                        )
                    nc.scalar.copy(out=xs[:, r, 0:gh, :], in_=ps[:, :gh, :])
                    ps = ppool.tile([128, GHMAX, V], mybir.dt.float32, tag="psB",
                                    bufs=2)
                    for q in range(NJ):
                        lhsT = fw_t[:, (r * NQ + q) * 128 : (r * NQ + q + 1) * 128]
                        rhs = yin[:, 0:gh, q : q + BS * (V - 1) + 1 : BS]
                        nc.tensor.matmul(
                            ps[:, :gh, :], lhsT, rhs,
                            start=(q == 0), stop=(q == NJ - 1),
                        )
                    nc.scalar.copy(out=ys[:, r, 0:gh, :], in_=ps[:, :gh, :])

                # deferred inverse: emit groups whose products finished in
                # prior chunks AFTER this chunk's forward matmuls, so the PE
                # queue never stalls waiting on the product engines
                while (
                    inv_emitted < len(_IGROUPS)
                    and _IGROUPS[inv_emitted][0] + _IGROUPS[inv_emitted][1] <= g0
                ):
                    emit_inverse(inv_emitted)
                    inv_emitted += 1

                # ---- pointwise products + v-sum tree (DVE + GpSimd) ----
                # bias the last chunk toward DVE (faster) to shrink the tail
                gpf, rdf = (1, 8) if last else (GP_FRAC, RED_FRAC)
                HV = V // 2
                for c in range(C):
                    for rh in range(NJ):
                        with nc.allow_low_precision("bf16 spectra products"):
                            pr = wpool.tile([128, GHMAX, 2, V], DT_VE, tag="pr",
                                            bufs=4)
                            tt_eng(gpf, rdf).tensor_mul(
                                pr[:, :gh, 0, :],
                                xs[:, rh, :gh, c : c + V],
                                ys[:, rh, :gh, :],
                            )
                            tt_eng(gpf, rdf).tensor_mul(
                                pr[:, :gh, 1, :],
                                xs[:, NJ + rh, :gh, c : c + V],
                                ys[:, NJ + rh, :gh, :],
                            )
                            # tree-sum over (2, V): stride-1 halves each pass
                            w4 = wpool.tile([128, GHMAX, 2, HV], DT_VE, tag="w4",
                                            bufs=4)
                            tt_eng(gpf, rdf).tensor_add(
                                w4[:, :gh], pr[:, :gh, :, 0:HV],
                                pr[:, :gh, :, HV:V],
                            )
                            w2 = wpool.tile([128, GHMAX, 2, HV // 2], DT_VE,
                                            tag="w2", bufs=4)
                            tt_eng(gpf, rdf).tensor_add(
                                w2[:, :gh], w4[:, :gh, :, 0 : HV // 2],
                                w4[:, :gh, :, HV // 2 : HV],
                            )
                            w1 = wpool.tile([128, GHMAX, 2], DT_VE, tag="w1",
                                            bufs=4)
                            tt_eng(gpf, rdf).tensor_add(
                                w1[:, :gh], w2[:, :gh, :, 0], w2[:, :gh, :, 1]
                            )
                            tt_eng(gpf, rdf).tensor_add(
                                zr[:, rh, g0 : g0 + gh, c], w1[:, :gh, 0],
                                w1[:, :gh, 1],
                            )
                            pr2 = wpool.tile([128, GHMAX, 2, V], DT_VE, tag="pr",
                                             bufs=4)
                            tt_eng(gpf, rdf).tensor_mul(
                                pr2[:, :gh, 0, :],
                                xs[:, NJ + rh, :gh, c : c + V],
                                ys[:, rh, :gh, :],
                            )
                            tt_eng(gpf, rdf).tensor_mul(
                                pr2[:, :gh, 1, :],
                                xs[:, rh, :gh, c : c + V],
                                ys[:, NJ + rh, :gh, :],
                            )
                            m4 = wpool.tile([128, GHMAX, 2, HV], DT_VE, tag="w4",
                                            bufs=4)
                            tt_eng(gpf, rdf).tensor_add(
                                m4[:, :gh], pr2[:, :gh, :, 0:HV],
                                pr2[:, :gh, :, HV:V],
                            )
                            m2 = wpool.tile([128, GHMAX, 2, HV // 2], DT_VE,
                                            tag="w2", bufs=4)
                            tt_eng(gpf, rdf).tensor_add(
                                m2[:, :gh], m4[:, :gh, :, 0 : HV // 2],
                                m4[:, :gh, :, HV // 2 : HV],
                            )
                            tt_eng(gpf, rdf).tensor_add(
                                zi[:, rh, g0 : g0 + gh, :, c], m2[:, :gh, :, 0],
                                m2[:, :gh, :, 1],
                            )

            while inv_emitted < len(_IGROUPS):
                emit_inverse(inv_emitted)
                inv_emitted += 1

    nc.compile()
    return nc


def _prep_core_inputs(d1f, d2f, fw, mt, core):
    """d1f/d2f: [2048, 3000] fp32. Returns the in_map for `core`."""
    sl = slice(core * G, (core + 1) * G)
    x = d1f[sl]
    y = d2f[sl]
    xp = np.zeros((G, NBB), dtype=np.float32)
    xp[:, SHIFT : SHIFT + NT] = x
    yp = np.zeros((G, V * B), dtype=np.float32)
    yp[:, :NT] = y
    # device layouts: xpT[p, g, u] = xp[g, 128u + p]
    xpT = np.ascontiguousarray(xp.reshape(G, U, 128).transpose(2, 0, 1)).astype(NP_MM)
    ypT = np.ascontiguousarray(yp.reshape(G, W, 128).transpose(2, 0, 1)).astype(NP_MM)
    return {"xp": xpT, "yp": ypT, "fw": fw.astype(NP_MM), "mt": mt.astype(NP_MM)}


def kernel(data1: np.ndarray, data2: np.ndarray) -> np.ndarray:
    import time

    d1f = np.ascontiguousarray(data1, dtype=np.float32).reshape(-1, NT)
    d2f = np.ascontiguousarray(data2, dtype=np.float32).reshape(-1, NT)
    fw, mt = _const_tiles()

    t0 = time.time()
    if "nc" not in _PE_CACHE:
        _PE_CACHE["nc"] = build_kernel()
    nc = _PE_CACHE["nc"]
    print(f"[kernel] build+compile {time.time() - t0:.1f}s", file=sys.stderr,
          flush=True)

    in_maps = [_prep_core_inputs(d1f, d2f, fw, mt, i) for i in range(NCORES)]
    t0 = time.time()
    res = run_bass_kernel_spmd(nc, in_maps, core_ids=list(range(NCORES)))
    print(f"[kernel] spmd run {time.time() - t0:.1f}s", file=sys.stderr, flush=True)
    global LAST_EXEC_NS, LAST_TRACE
    LAST_EXEC_NS = res.exec_time_ns
    LAST_TRACE = res.instructions_and_trace
    if res.exec_time_ns is not None:
        print(f"[kernel] HW exec {res.exec_time_ns} ns", file=sys.stderr, flush=True)

    outs = []
    for i in range(NCORES):
        o = res.results[i]["out"]  # [128, G, NJ, C]
        # out[g, B*c + 128*jg + p] = o[p, g, jg, c]
        full = o.transpose(1, 3, 2, 0).reshape(G, C * B)
        outs.append(full[:, :LAGS])
    return np.concatenate(outs, axis=0).reshape(NB_PAIRS, NCH, LAGS)

